# revision 1
# baseline (speedup 1.0000x reference)
"""Trainium2 Bass kernel for CombinedBandPassFilterSequential.

Zero-phase (filtfilt-style) FIR filter bank: 10 phase bands (K=769) +
10 amplitude bands (K=129) over a single (1,1,2097152) fp32 signal;
output is the 20 band signals concatenated on the last axis.

Strategy
--------
Time-sharded SPMD over 8 NeuronCores: each core processes a contiguous
T/8 slice of the signal for ALL 20 bands (perfect load balance).

Each 1-D correlation is cast as a sequence of 128x128 @ 128xN
tensor-engine matmuls using banded-Toeplitz weight chunks:

  out[128*i + r] = sum_q  W_q[:, r] . x_cols[:, i + q - Q0]

where x_cols[p, m] = x[128*m + p] is the signal in "transposed" column
layout (prepared on host) and W_q[p, r] = h[128*(q-Q0) + p - r + c].
The contraction (partition) dim is the tap offset; the moving dim packs
412-512 consecutive output blocks, so the PE runs dense matmuls at full
rate in float32r.

pha bands: two passes (corr with h, then with flip(h)) — exactly the
reference's zero-phase scheme; the 'SAME' zero-pad crop of the
intermediate at the global sequence edges is enforced via per-core 0/1
masks on its halo columns (one SPMD program serves all cores).

amp bands: single fused pass with g = autocorr(h) (257 taps), which
equals the two-pass result everywhere except the first/last 64 samples
of the GLOBAL sequence; those get an exact rank-64 correction
(precomputed 64x64 matrices applied to x's global head/tail, fed as a
per-core input that is zero except on cores 0/7 — again SPMD-uniform).
"""
import numpy as np

import concourse.bass as bass
import concourse.tile as tile
from concourse import bacc, mybir
from concourse import bass_utils

# ---- problem geometry (hardcoded per contest rules) ----
T = 2097152
NCORES = 8
L = T // NCORES          # 262144 samples per core
LC = L // 128            # 2048 output columns per core
XH = 8                   # x halo columns each side
XC = LC + 2 * XH         # 2064 x columns
YH = 4                   # y1 halo columns each side (pha)
YC = LC + 2 * YH         # 2056 y1 columns
NB = 10                  # bands per filter group
KP, QP, Q0P = 769, 7, 3  # pha: taps, Toeplitz chunks, chunk offset
KA = 129                 # amp taps
QG, Q0G = 3, 1           # fused amp autocorr (257 taps): chunks, offset
CA = (KA - 1) // 2       # 64: amp edge-correction width
P1N = 412                # pass-1 matmul moving width (ceil(2056/5))
P2N = 512                # pass-2 / fused matmul moving width

F32 = mybir.dt.float32
F32R = mybir.dt.float32r


def _toeplitz_chunks(h, Q0, NQ):
    """W[q][p, r] = h[128*(q - Q0) + p - r + c], zero outside [0, K)."""
    h = np.asarray(h, np.float64)
    K = len(h)
    c = (K - 1) // 2
    W = np.zeros((NQ, 128, 128), np.float64)
    p = np.arange(128)[:, None]
    r = np.arange(128)[None, :]
    for q in range(NQ):
        k = 128 * (q - Q0) + p - r + c
        valid = (k >= 0) & (k < K)
        W[q][valid] = h[np.clip(k, 0, K - 1)][valid]
    return W.astype(np.float32)


def _amp_corr_lhsT(h):
    """Block-diag [MleftT, MrightT] edge-correction matrix for one amp band.

    ref[n] = fused[n] - Mleft[n,:] @ x[:64]        for n in [0, 64)
    ref[n] = fused[n] - Mright[n-T+64,:] @ x[T-64:] for n in [T-64, T)
    """
    h = np.asarray(h, np.float64)
    K = len(h)
    c = (K - 1) // 2

    def hpad(idx):
        v = np.zeros(idx.shape)
        ok = (idx >= 0) & (idx < K)
        v[ok] = h[np.clip(idx, 0, K - 1)][ok]
        return v

    n = np.arange(c)[:, None, None]
    i = np.arange(c)[None, :, None]
    m = np.arange(-c, 0)[None, None, :]
    Mleft = (hpad(n + c - m) * hpad(i + c - m)).sum(-1)
    mm = np.arange(1, c + 1)[None, None, :]
    Mright = (hpad(n - mm + 1) * hpad(i - mm + 1)).sum(-1)

    lhsT = np.zeros((128, 128), np.float64)
    lhsT[:c, :c] = Mleft.T          # lhsT[p=i, r=n]
    lhsT[c:2 * c, c:2 * c] = Mright.T
    return lhsT.astype(np.float32)


def _build_program():
    nc = bacc.Bacc("TRN2", target_bir_lowering=False, debug=False,
                   enable_asserts=True, num_devices=NCORES)

    x_ap = nc.dram_tensor("xT", [128, XC], F32R, kind="ExternalInput").ap()
    wp1_ap = nc.dram_tensor("wp1", [128, NB * QP * 128], F32R,
                            kind="ExternalInput").ap()
    wp2_ap = nc.dram_tensor("wp2", [128, NB * QP * 128], F32R,
                            kind="ExternalInput").ap()
    wg_ap = nc.dram_tensor("wg", [128, NB * QG * 128], F32R,
                           kind="ExternalInput").ap()
    wc_ap = nc.dram_tensor("wc", [128, NB * 128], F32,
                           kind="ExternalInput").ap()
    xe_ap = nc.dram_tensor("xe", [128, 1], F32, kind="ExternalInput").ap()
    ml_ap = nc.dram_tensor("mask_l", [128, YH], F32R, kind="ExternalInput").ap()
    mr_ap = nc.dram_tensor("mask_r", [128, YH], F32R, kind="ExternalInput").ap()
    out_ap = nc.dram_tensor("out", [2 * NB, 128, LC], F32,
                            kind="ExternalOutput").ap()

    with tile.TileContext(nc) as tc:
        with tc.tile_pool(name="const", bufs=1) as cpool, \
             tc.tile_pool(name="y1", bufs=2) as y1_pool, \
             tc.tile_pool(name="psum", bufs=6, space="PSUM") as psum_pool, \
             tc.tile_pool(name="psumc", bufs=2, space="PSUM") as psumc_pool, \
             tc.tile_pool(name="stage", bufs=4) as stage_pool:

            xt = cpool.tile([128, XC], F32R, name="xt", tag="x")
            wp1 = cpool.tile([128, NB * QP * 128], F32R, name="wp1t", tag="wp1")
            wp2 = cpool.tile([128, NB * QP * 128], F32R, name="wp2t", tag="wp2")
            wg = cpool.tile([128, NB * QG * 128], F32R, name="wgt", tag="wg")
            wc = cpool.tile([128, NB * 128], F32, name="wct", tag="wc")
            xe = cpool.tile([128, 1], F32, name="xet", tag="xe")
            ml = cpool.tile([128, YH], F32R, name="mlt", tag="mask_l")
            mr = cpool.tile([128, YH], F32R, name="mrt", tag="mask_r")

            # DMAs ordered by first use: x + band-0 weights + masks first,
            # so the PE never waits on the bulk of the weight traffic.
            # xT lands in 4 chunks so pass1(0) group 0 only waits for the
            # columns it reads.
            # xT on the ACT HWDGE ring, weights on the SP ring — the two
            # rings drain in parallel, halving time-to-first-matmul
            for s in range(0, XC, 516):
                e = min(XC, s + 516)
                nc.scalar.dma_start(xt[:, s:e], x_ap[:, s:e])

            def wslice(tile_, ap, b, nq):
                s = b * nq * 128
                e = (b + 1) * nq * 128
                nc.sync.dma_start(tile_[:, s:e], ap[:, s:e])

            wslice(wp1, wp1_ap, 0, QP)
            nc.sync.dma_start(ml[:], ml_ap[:])
            nc.sync.dma_start(mr[:], mr_ap[:])
            wslice(wp2, wp2_ap, 0, QP)
            nc.sync.dma_start(xe[:], xe_ap[:])
            wslice(wg, wg_ap, 0, QG)
            wslice(wc, wc_ap, 0, 1)
            for b in range(1, NB):
                wslice(wp1, wp1_ap, b, QP)
                wslice(wp2, wp2_ap, b, QP)
                wslice(wg, wg_ap, b, QG)
                wslice(wc, wc_ap, b, 1)

            y1_tiles = [None] * NB

            def pha_pass1(b):
                y1 = y1_pool.tile([128, YC], F32R, tag="y1")
                y1_tiles[b] = y1
                for g in range(5):
                    j0 = g * P1N
                    n = min(P1N, YC - j0)
                    ps = psum_pool.tile([128, P2N], F32, tag="ps")
                    for q in range(QP):
                        m0 = j0 + q + XH - YH - Q0P
                        nc.tensor.matmul(
                            ps[:, :n],
                            wp1[:, (b * QP + q) * 128:(b * QP + q + 1) * 128],
                            xt[:, m0:m0 + n],
                            start=(q == 0), stop=(q == QP - 1),
                        )
                    nc.vector.tensor_copy(y1[:, j0:j0 + n], ps[:, :n])
                # 'SAME' crop of y1 outside the global [0, T) range
                nc.vector.tensor_mul(y1[:, :YH], y1[:, :YH], ml[:])
                nc.vector.tensor_mul(y1[:, YC - YH:], y1[:, YC - YH:], mr[:])

            def pha_pass2(b):
                y1 = y1_tiles[b]
                for g in range(LC // P2N):
                    i0 = g * P2N
                    ps = psum_pool.tile([128, P2N], F32, tag="ps")
                    for q in range(QP):
                        j0 = i0 + q - Q0P + YH
                        nc.tensor.matmul(
                            ps[:],
                            wp2[:, (b * QP + q) * 128:(b * QP + q + 1) * 128],
                            y1[:, j0:j0 + P2N],
                            start=(q == 0), stop=(q == QP - 1),
                        )
                    st = stage_pool.tile([128, P2N], F32, tag="st")
                    if g % 2 == 0:
                        nc.vector.tensor_copy(st[:], ps[:])
                    else:
                        nc.scalar.copy(st[:], ps[:])
                    nc.sync.dma_start(out_ap[b, :, i0:i0 + P2N], st[:])

            def amp_band(b):
                # rank-64 global-edge correction (zero on inner cores)
                pc = psumc_pool.tile([128, 1], F32, tag="pc")
                nc.tensor.matmul(pc[:], wc[:, b * 128:(b + 1) * 128], xe[:],
                                 start=True, stop=True)
                for g in range(LC // P2N):
                    i0 = g * P2N
                    ps = psum_pool.tile([128, P2N], F32, tag="ps")
                    for q in range(QG):
                        m0 = i0 + q - Q0G + XH
                        nc.tensor.matmul(
                            ps[:],
                            wg[:, (b * QG + q) * 128:(b * QG + q + 1) * 128],
                            xt[:, m0:m0 + P2N],
                            start=(q == 0), stop=(q == QG - 1),
                        )
                    st = stage_pool.tile([128, P2N], F32, tag="st")
                    # alternate PSUM->SBUF drains across DVE and ACT so the
                    # copies keep up with the 3-matmul amp groups
                    if g % 2 == 0:
                        nc.vector.tensor_copy(st[:], ps[:])
                    else:
                        nc.scalar.copy(st[:], ps[:])
                    if g == 0:
                        nc.vector.tensor_sub(st[:CA, :1], st[:CA, :1],
                                             pc[:CA, :])
                    if g == LC // P2N - 1:
                        nc.vector.tensor_sub(st[CA:2 * CA, P2N - 1:],
                                             st[CA:2 * CA, P2N - 1:],
                                             pc[CA:2 * CA, :])
                    nc.sync.dma_start(out_ap[NB + b, :, i0:i0 + P2N], st[:])

            # pha bands software-pipelined so the PE never waits on the
            # pass1 -> pass2 handoff; amp bands interleaved between pha
            # bands so their drain-heavy PSUM copies land in regions
            # where DVE/ACT otherwise have slack.
            pha_pass1(0)
            for b in range(NB):
                if b + 1 < NB:
                    pha_pass1(b + 1)
                pha_pass2(b)
                amp_band(b)

    nc.compile()
    return nc


_CACHE = {}


def _get_program():
    if "nc" not in _CACHE:
        _CACHE["nc"] = _build_program()
    return _CACHE["nc"]


def _host_inputs(x, pha_filters, amp_filters):
    x = np.ascontiguousarray(np.asarray(x, np.float32).reshape(T))
    pha = np.asarray(pha_filters, np.float32)
    amp = np.asarray(amp_filters, np.float32)

    wp1 = np.stack([_toeplitz_chunks(h, Q0P, QP) for h in pha])
    wp2 = np.stack([_toeplitz_chunks(h[::-1], Q0P, QP) for h in pha])
    gs = [np.correlate(np.asarray(h, np.float64),
                       np.asarray(h, np.float64), "full") for h in amp]
    wg = np.stack([_toeplitz_chunks(g, Q0G, QG) for g in gs])
    wc = np.stack([_amp_corr_lhsT(h) for h in amp])  # (NB, 128, 128)

    def wlay(W):  # (NB, NQ, 128p, 128r) -> (128p, NB*NQ*128r)
        return np.ascontiguousarray(W.transpose(2, 0, 1, 3).reshape(128, -1))

    wp1, wp2, wg = wlay(wp1), wlay(wp2), wlay(wg)
    wc = np.ascontiguousarray(wc.transpose(1, 0, 2).reshape(128, -1))

    xp = np.zeros(T + 2 * XH * 128, np.float32)
    xp[XH * 128: XH * 128 + T] = x

    ones = np.ones((128, YH), np.float32)
    zeros = np.zeros((128, YH), np.float32)
    xe0 = np.zeros((128, 1), np.float32)
    xe_head = xe0.copy()
    xe_head[:CA, 0] = x[:CA]
    xe_tail = xe0.copy()
    xe_tail[CA:2 * CA, 0] = x[T - CA:]

    in_maps = []
    for c in range(NCORES):
        n0 = c * L
        xT = np.ascontiguousarray(
            xp[n0:n0 + L + 2 * XH * 128].reshape(XC, 128).T)
        in_maps.append({
            "xT": xT,
            "wp1": wp1, "wp2": wp2, "wg": wg, "wc": wc,
            "xe": xe_head if c == 0 else (xe_tail if c == NCORES - 1 else xe0),
            "mask_l": zeros if c == 0 else ones,
            "mask_r": zeros if c == NCORES - 1 else ones,
        })
    return in_maps


def _gather(results):
    out = np.empty((2 * NB, T), np.float32)
    for c in range(NCORES):
        oc = results[c]["out"]  # (20, 128, LC): [band, r, i] = y[128*i + r]
        out[:, c * L:(c + 1) * L] = oc.transpose(0, 2, 1).reshape(2 * NB, L)
    return out.reshape(1, 1, 2 * NB * T)


def run(x, pha_filters, amp_filters, trace=False):
    nc = _get_program()
    in_maps = _host_inputs(x, pha_filters, amp_filters)
    res = bass_utils.run_bass_kernel_spmd(
        nc, in_maps, core_ids=list(range(NCORES)), trace=trace)
    return _gather(res.results), res


def kernel(x, pha_filters, amp_filters):
    out, _ = run(x, pha_filters, amp_filters)
    return out



# revision 2
# speedup vs baseline: 1.0335x; 1.0335x over previous
"""Trainium2 Bass kernel for CombinedBandPassFilterSequential.

Zero-phase (filtfilt-style) FIR filter bank: 10 phase bands (K=769) +
10 amplitude bands (K=129) over a single (1,1,2097152) fp32 signal;
output is the 20 band signals concatenated on the last axis.

Strategy
--------
Time-sharded SPMD over 8 NeuronCores: each core processes a contiguous
T/8 slice of the signal for ALL 20 bands (perfect load balance).

Every band is computed as a SINGLE fused correlation with the filter's
autocorrelation g = corr(h, h) (zero-phase transfer): pha bands get a
1537-tap g (13 Toeplitz chunks), amp bands a 257-tap g (3 chunks).
Each 1-D correlation is a sequence of 128x128 @ 128x512 tensor-engine
matmuls over banded-Toeplitz weight chunks in bf16 (fp32 PSUM accum):

  out[128*i + r] = sum_q  W_q[:, r] . x_cols[:, i + q - Q0]

where x_cols[p, m] = x[128*m + p] and W_q[p, r] = g[128*(q-Q0) + p - r + c].

The fused result differs from the reference's two-pass scheme only in
the first/last c = (K-1)/2 samples of the GLOBAL sequence (the 'SAME'
zero-pad crop of the intermediate).  Those 384 (pha) / 64 (amp) samples
per side get an exact precomputed correction, shipped as a per-core
input that is zero except on cores 0/7 (one SPMD program serves all
cores) and applied with two tiny vector subtracts per band.

A short burst of dummy warm-up matmuls runs while the first input DMAs
are in flight so the PE's HAM clock-gate is already at full rate when
real work arrives.
"""
import numpy as np
import ml_dtypes

import concourse.bass as bass
import concourse.tile as tile
from concourse import bacc, mybir
from concourse import bass_utils

# ---- problem geometry (hardcoded per contest rules) ----
T = 2097152
NCORES = 8
L = T // NCORES          # 262144 samples per core
LC = L // 128            # 2048 output columns per core
XH = 6                   # x halo columns each side (768 samples)
XC = LC + 2 * XH         # 2060 x columns
NB = 10                  # bands per filter group
QP, Q0P = 13, 6          # pha fused autocorr (1537 taps): chunks, offset
QA, Q0A = 3, 1           # amp fused autocorr (257 taps): chunks, offset
CP = 384                 # pha edge-correction width (3 cols)
CA = 64                  # amp edge-correction width
N = 512                  # matmul moving width
NG = LC // N             # 4 output groups per band
NWARM = 14               # warm-up matmuls

F32 = mybir.dt.float32
BF16 = mybir.dt.bfloat16
BFNP = ml_dtypes.bfloat16


def _toeplitz_chunks(h, Q0, NQ):
    """W[q][p, r] = h[128*(q - Q0) + p - r + c], zero outside [0, K)."""
    h = np.asarray(h, np.float64)
    K = len(h)
    c = (K - 1) // 2
    W = np.zeros((NQ, 128, 128), np.float64)
    p = np.arange(128)[:, None]
    r = np.arange(128)[None, :]
    for q in range(NQ):
        k = 128 * (q - Q0) + p - r + c
        valid = (k >= 0) & (k < K)
        W[q][valid] = h[np.clip(k, 0, K - 1)][valid]
    return W


def _head_D(h, xh):
    """fused - exact on the first c samples; xh = x[:3c] (float64).

    exact: two-pass zero-phase with the intermediate cropped to [0, T)
    (the reference's 'SAME' scheme); fused: correlation with autocorr(h).
    """
    h = np.asarray(h, np.float64)
    K = len(h)
    c = (K - 1) // 2
    xp = np.concatenate([np.zeros(c), xh])           # xp[m] = x[m - c]
    y1 = np.correlate(xp, h, 'valid')                # y1[t], t in [0, 2c)
    z = np.concatenate([np.zeros(c), y1])            # z[m] = y1[m - c]
    yex = np.convolve(h, z)[2 * c: 3 * c]            # exact y[0:c]
    g = np.correlate(h, h, 'full')                   # 2K-1 taps
    xq = np.concatenate([np.zeros(2 * c), xh])       # xq[m] = x[m - 2c]
    f = np.correlate(xq, g, 'valid')[:c]             # fused y[0:c]
    return f - yex


def _build_program():
    nc = bacc.Bacc("TRN2", target_bir_lowering=False, debug=False,
                   enable_asserts=True, num_devices=NCORES)

    x_ap = nc.dram_tensor("xT", [128, XC], BF16, kind="ExternalInput").ap()
    wp_ap = nc.dram_tensor("wp", [128, NB * QP * 128], BF16,
                           kind="ExternalInput").ap()
    wa_ap = nc.dram_tensor("wa", [128, NB * QA * 128], BF16,
                           kind="ExternalInput").ap()
    ch_ap = nc.dram_tensor("corr_h", [128, 2 * NB * 3], F32,
                           kind="ExternalInput").ap()
    ct_ap = nc.dram_tensor("corr_t", [128, 2 * NB * 3], F32,
                           kind="ExternalInput").ap()
    out_ap = nc.dram_tensor("out", [2 * NB, 128, LC], F32,
                            kind="ExternalOutput").ap()

    with tile.TileContext(nc) as tc:
        with tc.tile_pool(name="const", bufs=1) as cpool, \
             tc.tile_pool(name="psum", bufs=6, space="PSUM") as psum_pool, \
             tc.tile_pool(name="psumw", bufs=1, space="PSUM") as psumw_pool, \
             tc.tile_pool(name="stage", bufs=4) as stage_pool:

            xt = cpool.tile([128, XC], BF16, name="xt", tag="x")
            wp = cpool.tile([128, NB * QP * 128], BF16, name="wpt", tag="wp")
            wa = cpool.tile([128, NB * QA * 128], BF16, name="wat", tag="wa")
            ch = cpool.tile([128, 2 * NB * 3], F32, name="cht", tag="ch")
            ct = cpool.tile([128, 2 * NB * 3], F32, name="ctt", tag="ct")
            wrm = cpool.tile([128, 256], BF16, name="wrm", tag="warm")

            # ---- PE warm-up while the first input DMAs are in flight ----
            nc.vector.memset(wrm[:], 0.0)
            pw = psumw_pool.tile([128, 256], F32, tag="pw")
            for _ in range(NWARM):
                nc.tensor.matmul(pw[:], wrm[:, :128], wrm[:],
                                 start=True, stop=True)

            # ---- input DMAs, ordered by first use ----
            # sync (SP) ring: first x chunk, all amp weights, rest of x,
            # corrections.  scalar (ACT) ring: pha weights band by band.
            nc.sync.dma_start(xt[:, 0:528], x_ap[:, 0:528])
            nc.sync.dma_start(wa[:], wa_ap[:])
            for s in range(528, XC, 512):
                e = min(XC, s + 512)
                nc.sync.dma_start(xt[:, s:e], x_ap[:, s:e])
            nc.sync.dma_start(ch[:], ch_ap[:])
            nc.sync.dma_start(ct[:], ct_ap[:])
            for b in range(NB):
                s = b * QP * 128
                e = (b + 1) * QP * 128
                nc.scalar.dma_start(wp[:, s:e], wp_ap[:, s:e])

            ncopy = [0]

            def drain(ps, band_out, g, cb, cbt):
                """PSUM -> SBUF (alternating DVE/ACT), edge-fix, DMA out."""
                st = stage_pool.tile([128, N], F32, tag="st")
                if ncopy[0] % 2 == 0:
                    nc.vector.tensor_copy(st[:], ps[:])
                else:
                    nc.scalar.copy(st[:], ps[:])
                ncopy[0] += 1
                if g == 0:
                    nc.vector.tensor_sub(st[:, 0:3], st[:, 0:3],
                                         ch[:, cb:cb + 3])
                if g == NG - 1:
                    nc.vector.tensor_sub(st[:, N - 3:], st[:, N - 3:],
                                         ct[:, cbt:cbt + 3])
                nc.sync.dma_start(out_ap[band_out, :, g * N:(g + 1) * N],
                                  st[:])

            # ---- amp bands first (small weights -> earliest start);
            # g-outer so group 0 only needs the first x chunk ----
            for g in range(NG):
                for b in range(NB):
                    ps = psum_pool.tile([128, N], F32, tag="ps")
                    for q in range(QA):
                        m0 = XH + g * N + q - Q0A
                        nc.tensor.matmul(
                            ps[:],
                            wa[:, (b * QA + q) * 128:(b * QA + q + 1) * 128],
                            xt[:, m0:m0 + N],
                            start=(q == 0), stop=(q == QA - 1),
                        )
                    cb = (NB + b) * 3
                    drain(ps, NB + b, g, cb, cb)

            # ---- pha bands, band-outer (weights stream in per band) ----
            for b in range(NB):
                for g in range(NG):
                    ps = psum_pool.tile([128, N], F32, tag="ps")
                    for q in range(QP):
                        m0 = XH + g * N + q - Q0P
                        nc.tensor.matmul(
                            ps[:],
                            wp[:, (b * QP + q) * 128:(b * QP + q + 1) * 128],
                            xt[:, m0:m0 + N],
                            start=(q == 0), stop=(q == QP - 1),
                        )
                    drain(ps, b, g, b * 3, b * 3)

    nc.compile()
    return nc


_CACHE = {}


def _get_program():
    if "nc" not in _CACHE:
        _CACHE["nc"] = _build_program()
    return _CACHE["nc"]


def _host_inputs(x, pha_filters, amp_filters):
    x = np.ascontiguousarray(np.asarray(x, np.float32).reshape(T))
    pha = np.asarray(pha_filters, np.float64)
    amp = np.asarray(amp_filters, np.float64)

    gp = [np.correlate(h, h, 'full') for h in pha]   # 1537 taps
    ga = [np.correlate(h, h, 'full') for h in amp]   # 257 taps
    wp = np.stack([_toeplitz_chunks(g, Q0P, QP) for g in gp])
    wa = np.stack([_toeplitz_chunks(g, Q0A, QA) for g in ga])

    def wlay(W):  # (NB, NQ, 128p, 128r) -> (128p, NB*NQ*128r) bf16
        return np.ascontiguousarray(
            W.transpose(2, 0, 1, 3).reshape(128, -1).astype(BFNP))

    wp, wa = wlay(wp), wlay(wa)

    x64 = x.astype(np.float64)
    # edge corrections: D = fused - exact (fp64), head and tail per band
    ch = np.zeros((128, 2 * NB * 3), np.float32)
    ct = np.zeros((128, 2 * NB * 3), np.float32)
    for b in range(NB):
        dh = _head_D(pha[b], x64[:3 * CP])
        dt = _head_D(pha[b][::-1], x64[T - 3 * CP:][::-1])[::-1]
        ch[:, 3 * b:3 * b + 3] = dh.reshape(3, 128).T
        ct[:, 3 * b:3 * b + 3] = dt.reshape(3, 128).T
    for b in range(NB):
        dh = _head_D(amp[b], x64[:3 * CA])          # (64,)
        dt = _head_D(amp[b][::-1], x64[T - 3 * CA:][::-1])[::-1]
        ch[:CA, 3 * (NB + b)] = dh
        ct[CA:2 * CA, 3 * (NB + b) + 2] = dt
    zeros = np.zeros_like(ch)

    xp = np.zeros(T + 2 * XH * 128, np.float32)
    xp[XH * 128: XH * 128 + T] = x

    in_maps = []
    for c in range(NCORES):
        n0 = c * L
        xT = np.ascontiguousarray(
            xp[n0:n0 + L + 2 * XH * 128].reshape(XC, 128).T.astype(BFNP))
        in_maps.append({
            "xT": xT,
            "wp": wp, "wa": wa,
            "corr_h": ch if c == 0 else zeros,
            "corr_t": ct if c == NCORES - 1 else zeros,
        })
    return in_maps


def _gather(results):
    out = np.empty((2 * NB, T), np.float32)
    for c in range(NCORES):
        oc = results[c]["out"]  # (20, 128, LC): [band, r, i] = y[128*i + r]
        out[:, c * L:(c + 1) * L] = oc.transpose(0, 2, 1).reshape(2 * NB, L)
    return out.reshape(1, 1, 2 * NB * T)


def run(x, pha_filters, amp_filters, trace=False):
    nc = _get_program()
    in_maps = _host_inputs(x, pha_filters, amp_filters)
    res = bass_utils.run_bass_kernel_spmd(
        nc, in_maps, core_ids=list(range(NCORES)), trace=trace)
    return _gather(res.results), res


def kernel(x, pha_filters, amp_filters):
    out, _ = run(x, pha_filters, amp_filters)
    return out


# revision 5
# speedup vs baseline: 1.1710x; 1.1331x over previous
"""Trainium2 Bass kernel for CombinedBandPassFilterSequential.

Zero-phase (filtfilt-style) FIR filter bank: 10 phase bands (K=769) +
10 amplitude bands (K=129) over a single (1,1,2097152) fp32 signal;
output is the 20 band signals concatenated on the last axis.

Strategy
--------
Time-sharded SPMD over 8 NeuronCores: each core processes a contiguous
T/8 slice of the signal for ALL 20 bands (perfect load balance).

Every band is computed as a SINGLE fused correlation with the filter's
autocorrelation g = corr(h, h) (zero-phase transfer): pha bands get a
1537-tap g (13 Toeplitz chunks), amp bands a 257-tap g (3 chunks).
Each 1-D correlation is a sequence of 128x128 @ 128x512 tensor-engine
matmuls over banded-Toeplitz weight chunks in bf16 (fp32 PSUM accum):

  out[128*i + r] = sum_q  W_q[:, r] . x_cols[:, i + q - Q0]

where x_cols[p, m] = x[128*m + p] and W_q[p, r] = g[128*(q-Q0) + p - r + c].

The fused result differs from the reference's two-pass scheme only in
the first/last c = (K-1)/2 samples of the GLOBAL sequence (the 'SAME'
zero-pad crop of the intermediate).  Those 384 (pha) / 64 (amp) samples
per side get an exact precomputed correction, shipped as a per-core
input that is zero except on cores 0/7 (one SPMD program serves all
cores) and applied with two tiny vector subtracts per band.

A short burst of dummy warm-up matmuls runs while the first input DMAs
are in flight so the PE's HAM clock-gate is already at full rate when
real work arrives.
"""
import numpy as np
import ml_dtypes

import concourse.bass as bass
import concourse.tile as tile
from concourse import bacc, mybir
from concourse import bass_utils

# ---- problem geometry (hardcoded per contest rules) ----
T = 2097152
NCORES = 8
L = T // NCORES          # 262144 samples per core
LC = L // 128            # 2048 output columns per core
XH = 6                   # x halo columns each side (768 samples)
XC = LC + 2 * XH         # 2060 x columns
NB = 10                  # bands per filter group
QP, Q0P = 13, 6          # pha fused autocorr (1537 taps): chunks, offset
QA, Q0A = 3, 1           # amp fused autocorr (257 taps): chunks, offset
CP = 384                 # pha edge-correction width (3 cols)
CA = 64                  # amp edge-correction width
N = 512                  # matmul moving width
NG = LC // N             # 4 output groups per band
NWARM = 6                # warm-up matmuls

F32 = mybir.dt.float32
BF16 = mybir.dt.bfloat16
BFNP = ml_dtypes.bfloat16


def _toeplitz_chunks(h, Q0, NQ):
    """W[q][p, r] = h[128*(q - Q0) + p - r + c], zero outside [0, K)."""
    h = np.asarray(h, np.float64)
    K = len(h)
    c = (K - 1) // 2
    W = np.zeros((NQ, 128, 128), np.float64)
    p = np.arange(128)[:, None]
    r = np.arange(128)[None, :]
    for q in range(NQ):
        k = 128 * (q - Q0) + p - r + c
        valid = (k >= 0) & (k < K)
        W[q][valid] = h[np.clip(k, 0, K - 1)][valid]
    return W


def _head_D(h, xh):
    """fused - exact on the first c samples; xh = x[:3c] (float64).

    exact: two-pass zero-phase with the intermediate cropped to [0, T)
    (the reference's 'SAME' scheme); fused: correlation with autocorr(h).
    """
    h = np.asarray(h, np.float64)
    K = len(h)
    c = (K - 1) // 2
    xp = np.concatenate([np.zeros(c), xh])           # xp[m] = x[m - c]
    y1 = np.correlate(xp, h, 'valid')                # y1[t], t in [0, 2c)
    z = np.concatenate([np.zeros(c), y1])            # z[m] = y1[m - c]
    yex = np.convolve(h, z)[2 * c: 3 * c]            # exact y[0:c]
    g = np.correlate(h, h, 'full')                   # 2K-1 taps
    xq = np.concatenate([np.zeros(2 * c), xh])       # xq[m] = x[m - 2c]
    f = np.correlate(xq, g, 'valid')[:c]             # fused y[0:c]
    return f - yex


def _build_program():
    nc = bacc.Bacc("TRN2", target_bir_lowering=False, debug=False,
                   enable_asserts=True, num_devices=NCORES)

    x_ap = nc.dram_tensor("xT", [128, XC], BF16, kind="ExternalInput").ap()
    wp_ap = nc.dram_tensor("wp", [128, NB * QP * 128], BF16,
                           kind="ExternalInput").ap()
    wa_ap = nc.dram_tensor("wa", [128, NB * QA * 128], BF16,
                           kind="ExternalInput").ap()
    ch_ap = nc.dram_tensor("corr_h", [128, 2 * NB * 3], F32,
                           kind="ExternalInput").ap()
    ct_ap = nc.dram_tensor("corr_t", [128, 2 * NB * 3], F32,
                           kind="ExternalInput").ap()
    out_ap = nc.dram_tensor("out", [2 * NB, 128, LC], F32,
                            kind="ExternalOutput").ap()

    with tile.TileContext(nc) as tc:
        with tc.tile_pool(name="const", bufs=1) as cpool, \
             tc.tile_pool(name="psum", bufs=6, space="PSUM") as psum_pool, \
             tc.tile_pool(name="psumw", bufs=1, space="PSUM") as psumw_pool, \
             tc.tile_pool(name="stage", bufs=4) as stage_pool:

            xt = cpool.tile([128, XC], BF16, name="xt", tag="x")
            wp = cpool.tile([128, NB * QP * 128], BF16, name="wpt", tag="wp")
            wa = cpool.tile([128, NB * QA * 128], BF16, name="wat", tag="wa")
            ch = cpool.tile([128, 2 * NB * 3], F32, name="cht", tag="ch")
            ct = cpool.tile([128, 2 * NB * 3], F32, name="ctt", tag="ct")
            wrm = cpool.tile([128, N], BF16, name="wrm", tag="warm")

            # ---- PE warm-up while the first input DMAs are in flight ----
            nc.vector.memset(wrm[:], 0.0)
            pw = psumw_pool.tile([128, N], F32, tag="pw")
            for _ in range(NWARM):
                nc.tensor.matmul(pw[:], wrm[:, :128], wrm[:],
                                 start=True, stop=True)

            # ---- input DMAs, ordered by first use ----
            # sync (SP) ring: x chunks and per-band amp weights interleaved,
            # then corrections.  scalar (ACT) ring: pha weights band by band.
            def wa_slice(b):
                s, e = b * QA * 128, (b + 1) * QA * 128
                nc.sync.dma_start(wa[:, s:e], wa_ap[:, s:e])

            nc.sync.dma_start(xt[:, 0:528], x_ap[:, 0:528])
            wa_slice(0)
            nc.sync.dma_start(xt[:, 528:1040], x_ap[:, 528:1040])
            wa_slice(1)
            wa_slice(2)
            nc.sync.dma_start(xt[:, 1040:1552], x_ap[:, 1040:1552])
            for b in range(3, 6):
                wa_slice(b)
            nc.sync.dma_start(xt[:, 1552:XC], x_ap[:, 1552:XC])
            for b in range(6, NB):
                wa_slice(b)
            nc.sync.dma_start(ch[:], ch_ap[:])
            nc.sync.dma_start(ct[:], ct_ap[:])
            for b in range(NB):
                s = b * QP * 128
                e = (b + 1) * QP * 128
                nc.scalar.dma_start(wp[:, s:e], wp_ap[:, s:e])

            ncopy = [0]

            def drain(ps, band_out, g, cb, cbt):
                """PSUM -> SBUF (alternating DVE/ACT), edge-fix, DMA out."""
                st = stage_pool.tile([128, N], F32, tag="st")
                if ncopy[0] % 2 == 0:
                    nc.vector.tensor_copy(st[:], ps[:])
                else:
                    nc.scalar.copy(st[:], ps[:])
                ncopy[0] += 1
                if g == 0:
                    nc.vector.tensor_sub(st[:, 0:3], st[:, 0:3],
                                         ch[:, cb:cb + 3])
                if g == NG - 1:
                    nc.vector.tensor_sub(st[:, N - 3:], st[:, N - 3:],
                                         ct[:, cbt:cbt + 3])
                nc.sync.dma_start(out_ap[band_out, :, g * N:(g + 1) * N],
                                  st[:])

            def amp_group(b, g):
                ps = psum_pool.tile([128, N], F32, tag="ps")
                for q in range(QA):
                    m0 = XH + g * N + q - Q0A
                    nc.tensor.matmul(
                        ps[:],
                        wa[:, (b * QA + q) * 128:(b * QA + q + 1) * 128],
                        xt[:, m0:m0 + N],
                        start=(q == 0), stop=(q == QA - 1),
                    )
                drain(ps, NB + b, g, (NB + b) * 3, (NB + b) * 3)

            def pha_group(b, g):
                ps = psum_pool.tile([128, N], F32, tag="ps")
                for q in range(QP):
                    m0 = XH + g * N + q - Q0P
                    nc.tensor.matmul(
                        ps[:],
                        wp[:, (b * QP + q) * 128:(b * QP + q + 1) * 128],
                        xt[:, m0:m0 + N],
                        start=(q == 0), stop=(q == QP - 1),
                    )
                drain(ps, b, g, b * 3, b * 3)

            # ---- strict 1:1 interleave: each short (3-matmul) amp group
            # drains inside the next long (13-matmul) pha accumulation, so
            # PSUM drains never gate the PE ----
            amp_group(0, 0)
            for idx in range(NB * NG):
                pha_group(idx // NG, idx % NG)
                if idx + 1 < NB * NG:
                    amp_group((idx + 1) // NG, (idx + 1) % NG)

    nc.compile()
    return nc


_CACHE = {}


def _get_program():
    if "nc" not in _CACHE:
        _CACHE["nc"] = _build_program()
    return _CACHE["nc"]


def _host_inputs(x, pha_filters, amp_filters):
    x = np.ascontiguousarray(np.asarray(x, np.float32).reshape(T))
    pha = np.asarray(pha_filters, np.float64)
    amp = np.asarray(amp_filters, np.float64)

    gp = [np.correlate(h, h, 'full') for h in pha]   # 1537 taps
    ga = [np.correlate(h, h, 'full') for h in amp]   # 257 taps
    wp = np.stack([_toeplitz_chunks(g, Q0P, QP) for g in gp])
    wa = np.stack([_toeplitz_chunks(g, Q0A, QA) for g in ga])

    def wlay(W):  # (NB, NQ, 128p, 128r) -> (128p, NB*NQ*128r) bf16
        return np.ascontiguousarray(
            W.transpose(2, 0, 1, 3).reshape(128, -1).astype(BFNP))

    wp, wa = wlay(wp), wlay(wa)

    x64 = x.astype(np.float64)
    # edge corrections: D = fused - exact (fp64), head and tail per band
    ch = np.zeros((128, 2 * NB * 3), np.float32)
    ct = np.zeros((128, 2 * NB * 3), np.float32)
    for b in range(NB):
        dh = _head_D(pha[b], x64[:3 * CP])
        dt = _head_D(pha[b][::-1], x64[T - 3 * CP:][::-1])[::-1]
        ch[:, 3 * b:3 * b + 3] = dh.reshape(3, 128).T
        ct[:, 3 * b:3 * b + 3] = dt.reshape(3, 128).T
    for b in range(NB):
        dh = _head_D(amp[b], x64[:3 * CA])          # (64,)
        dt = _head_D(amp[b][::-1], x64[T - 3 * CA:][::-1])[::-1]
        ch[:CA, 3 * (NB + b)] = dh
        ct[CA:2 * CA, 3 * (NB + b) + 2] = dt
    zeros = np.zeros_like(ch)

    xp = np.zeros(T + 2 * XH * 128, np.float32)
    xp[XH * 128: XH * 128 + T] = x

    in_maps = []
    for c in range(NCORES):
        n0 = c * L
        xT = np.ascontiguousarray(
            xp[n0:n0 + L + 2 * XH * 128].reshape(XC, 128).T.astype(BFNP))
        in_maps.append({
            "xT": xT,
            "wp": wp, "wa": wa,
            "corr_h": ch if c == 0 else zeros,
            "corr_t": ct if c == NCORES - 1 else zeros,
        })
    return in_maps


def _gather(results):
    out = np.empty((2 * NB, T), np.float32)
    for c in range(NCORES):
        oc = results[c]["out"]  # (20, 128, LC): [band, r, i] = y[128*i + r]
        out[:, c * L:(c + 1) * L] = oc.transpose(0, 2, 1).reshape(2 * NB, L)
    return out.reshape(1, 1, 2 * NB * T)


def run(x, pha_filters, amp_filters, trace=False):
    nc = _get_program()
    in_maps = _host_inputs(x, pha_filters, amp_filters)
    res = bass_utils.run_bass_kernel_spmd(
        nc, in_maps, core_ids=list(range(NCORES)), trace=trace)
    return _gather(res.results), res


def kernel(x, pha_filters, amp_filters):
    out, _ = run(x, pha_filters, amp_filters)
    return out
